# revision 16
# baseline (speedup 1.0000x reference)
"""Trainium2 Bass kernel for nn_AttentionLayer (DIN-style attention scorer).

Math (per batch b):
  info[t] = [q, k[t], q-k[t], q*k[t]]  (256 feats)
  h0 = relu(info @ W0 + b0); h1 = relu(h0 @ W1 + b1); logit[t] = h1 @ Wf + bf
  att = softmax(mask ? logit : NEG); out = sum_t att[t] * v[t]

Key restructuring:
  info @ W0 = q@(W0a+W0c) + k@(W0b-W0c) + (q*k)@W0d
  -> one K=128 matmul over [k ; q*k] features (host-precomputed, transposed)
     plus one K=65 accumulating matmul with q broadcast over t via a step-0
     AP (row 64 of the stationary carries b0, against a host ones row).
  bf is dropped: a uniform logit shift is softmax-invariant. The softmax max
  subtraction is dropped too: logits are O(3) here, exp() is safe in f32,
  and masked lanes sit at NEG -> exp gives exactly 0.
  Softmax runs in [batch-partition, t-free] layout; att is PE-transposed to
  [t-partition, batch] so the weighted v-sum becomes per-batch K=128/K=72
  accumulating matmuls with a 1-column stationary.
  PSUM cannot be DMA'd, so small outputs (logits [1,400], wsum [1,64]) are
  packed across psum partitions {0,32,64,96} via tile_position col groups
  and evacuated with one wide DVE/ACT copy, then partition-strided DMA.

Sharding: batch 4096 -> 8 cores x 512. SPMD, no collectives.
"""

import numpy as np
import ml_dtypes

B_TOT, T, D = 4096, 200, 64
H0, H1 = 128, 64
NCORES = 8
BC = B_TOT // NCORES          # 512 batches per core
N = BC * T                    # 102400 (b,t) rows per core
TILE = 400                    # 2 batches per tile
NTILES = N // TILE            # 256
BT = 128                      # batches per B-tile (softmax block)
NBT = BC // BT                # 4
NEG = float(-(2**32) + 1)

bf16 = ml_dtypes.bfloat16

_BUILT = {}


def _build_program():
    import concourse.bacc as bacc
    import concourse.tile as tile
    from concourse import mybir

    fp32 = mybir.dt.float32
    bfl = mybir.dt.bfloat16
    AF = mybir.ActivationFunctionType
    ALU = mybir.AluOpType

    nc = bacc.Bacc("TRN2", target_bir_lowering=False, debug=False,
                   num_devices=NCORES)

    featD = nc.dram_tensor("feat", [128, N], bfl, kind="ExternalInput").ap()
    qbD = nc.dram_tensor("qb", [65, BC], bfl, kind="ExternalInput").ap()
    vvD = nc.dram_tensor("vv", [BC, T, D], bfl, kind="ExternalInput").ap()
    maD = nc.dram_tensor("maskadd", [BC, T], fp32, kind="ExternalInput").ap()
    w0D = nc.dram_tensor("w0", [128, 128], bfl, kind="ExternalInput").ap()
    wAD = nc.dram_tensor("wA", [65, 128], bfl, kind="ExternalInput").ap()
    w1D = nc.dram_tensor("w1", [128, 64], bfl, kind="ExternalInput").ap()
    wfD = nc.dram_tensor("wf2", [128, 1], bfl, kind="ExternalInput").ap()
    b1D = nc.dram_tensor("b1r", [128, 1], fp32, kind="ExternalInput").ap()
    idD = nc.dram_tensor("ident", [128, 128], bfl, kind="ExternalInput").ap()
    oD = nc.dram_tensor("o", [BC, D], fp32, kind="ExternalOutput").ap()
    lgD = nc.dram_tensor("lgscratch", [BC, T], fp32).ap()

    with tile.TileContext(nc) as tc:
        with (
            tc.tile_pool(name="wts", bufs=1) as wpool,
            tc.tile_pool(name="feat", bufs=8) as fpool,
            tc.tile_pool(name="h0", bufs=4) as h0pool,
            tc.tile_pool(name="h1", bufs=3) as h1pool,
            tc.tile_pool(name="lgsc", bufs=4) as scpool,
            tc.tile_pool(name="soft", bufs=3) as spool,
            tc.tile_pool(name="stat", bufs=4) as stpool,
            tc.tile_pool(name="vbuf", bufs=2) as vpool,
            tc.tile_pool(name="attT", bufs=2) as apool,
            tc.tile_pool(name="osb", bufs=2) as opool,
            tc.tile_pool(name="p0", bufs=3, space="PSUM") as p0pool,
            tc.tile_pool(name="p1", bufs=2, space="PSUM") as p1pool,
            tc.tile_pool(name="plg", bufs=1, space="PSUM") as lgpool,
            tc.tile_pool(name="paux", bufs=2, space="PSUM") as auxpool,
        ):
            w0_sb = wpool.tile([128, 128], bfl, tag="w0")
            nc.sync.dma_start(out=w0_sb[:], in_=w0D)
            wA_sb = wpool.tile([65, 128], bfl, tag="wA")
            nc.sync.dma_start(out=wA_sb[:], in_=wAD)
            w1_sb = wpool.tile([128, 64], bfl, tag="w1")
            nc.sync.dma_start(out=w1_sb[:], in_=w1D)
            wf_sb = wpool.tile([128, 1], bfl, tag="wf")
            nc.sync.dma_start(out=wf_sb[:], in_=wfD)
            b1_sb = wpool.tile([128, 1], fp32, tag="b1")
            nc.sync.dma_start(out=b1_sb[:], in_=b1D)
            id_sb = wpool.tile([128, 128], bfl, tag="ident")
            nc.sync.dma_start(out=id_sb[:], in_=idD)
            qb_sb = wpool.tile([65, BC], bfl, tag="qb")
            nc.sync.dma_start(out=qb_sb[:], in_=qbD)

            def mlp_block(bt):
                b0g = bt * BT
                ps1 = None
                h1_pair = [None, None]
                for j in range(BT * T // TILE):  # 64 tiles of 400 cols
                    i = bt * 64 + j
                    n0 = i * TILE
                    ft = fpool.tile([128, TILE], bfl, tag="ft")
                    nc.sync.dma_start(out=ft[:], in_=featD[:, n0:n0 + TILE])

                    ps0 = p0pool.tile([128, TILE], fp32, tag="ps0")
                    nc.tensor.matmul(ps0[:], w0_sb[:], ft[:],
                                     start=True, stop=False)
                    qsl = qb_sb[:, 2 * i:2 * i + 2].unsqueeze(2)
                    qb_bc = qsl.broadcast_to([65, 2, T])
                    ps0_3 = ps0[:].rearrange("p (b t) -> p b t", t=T)
                    nc.tensor.matmul(ps0_3, wA_sb[:], qb_bc,
                                     start=False, stop=True)

                    h0t = h0pool.tile([128, TILE], bfl, tag="h0")
                    if i % 2 == 0:
                        nc.scalar.activation(h0t[:], ps0[:], AF.Relu)
                    else:
                        nc.vector.tensor_scalar_max(h0t[:], ps0[:], 0.0)

                    # mm1: pack tile pairs into one [128, TILE] psum via
                    # column tiling; relu1 then covers two tiles at once.
                    if j % 2 == 0:
                        ps1 = p1pool.tile([128, TILE], fp32, tag="ps1")
                        nc.tensor.matmul(ps1[0:64, :], w1_sb[:], h0t[:],
                                         start=True, stop=True,
                                         tile_position=(0, 0))
                    else:
                        nc.tensor.matmul(ps1[64:128, :], w1_sb[:], h0t[:],
                                         start=True, stop=True,
                                         tile_position=(0, 64))
                        h1t = h1pool.tile([128, TILE], bfl, tag="h1")
                        if (j // 2) % 2 == 0:
                            nc.scalar.activation(h1t[:], ps1[:], AF.Relu,
                                                 bias=b1_sb[:])
                        else:
                            nc.vector.tensor_scalar(h1t[:], ps1[:],
                                                    b1_sb[:], 0.0,
                                                    ALU.add, ALU.max)
                        h1_pair[(j // 2) % 2] = h1t

                    # mm2 for a quad (2 pairs): logits to psum partitions
                    # {0,32,64,96} via row+col tile positions.
                    if j % 4 == 3:
                        lg_ps = lgpool.tile([128, TILE], fp32, tag="lg")
                        for sub in range(4):
                            hp = h1_pair[sub // 2]
                            r0 = (sub % 2) * 64
                            pp = sub * 32
                            nc.tensor.matmul(
                                lg_ps[pp:pp + 1, :],
                                wf_sb[r0:r0 + 64, :],
                                hp[r0:r0 + 64, :],
                                start=True, stop=True,
                                tile_position=(r0, pp))
                        sc = scpool.tile([128, TILE], fp32, tag="sc")
                        if (j // 4) % 2 == 0:
                            nc.scalar.copy(sc[:], lg_ps[:])
                        else:
                            nc.vector.tensor_copy(sc[:], lg_ps[:])
                        # 8 batches of logits -> DRAM scratch (SBUF dst
                        # cannot take a split partition dim; DRAM can)
                        bq = b0g + (j // 4) * 8
                        src = sc[0:128:32, :].rearrange(
                            "p (b t) -> p b t", t=T)
                        dst = lgD[bq:bq + 8, :].rearrange(
                            "(p b) t -> p b t", b=2)
                        nc.sync.dma_start(out=dst, in_=src)

            def tail_block(bt):
                b0g = bt * BT
                # ---- v tiles prefetch: [t-part, (b,d)] ----
                v1 = vpool.tile([128, BT * D], bfl, tag="v1")
                src1 = vvD[b0g:b0g + BT, 0:128, :].transpose([1, 0, 2])
                nc.sync.dma_start(
                    out=v1[:].rearrange("p (b d) -> p b d", d=D), in_=src1)
                v2 = vpool.tile([128, BT * D], bfl, tag="v2")
                src2 = vvD[b0g:b0g + BT, 128:200, :].transpose([1, 0, 2])
                nc.sync.dma_start(
                    out=v2[0:72, :].rearrange("p (b d) -> p b d", d=D),
                    in_=src2)
                # ---- softmax over T for this B-tile (no max needed:
                # logits are O(3); masked lanes NEG -> exp = 0) ----
                logit_sb = spool.tile([128, T], fp32, tag="lgsb")
                nc.sync.dma_start(out=logit_sb[:], in_=lgD[b0g:b0g + BT, :])
                madd = spool.tile([128, T], fp32, tag="madd")
                nc.sync.dma_start(out=madd[:], in_=maD[b0g:b0g + BT, :])
                lm = spool.tile([128, T], fp32, tag="lm")
                nc.vector.tensor_add(lm[:], logit_sb[:], madd[:])
                e = spool.tile([128, T], bfl, tag="e")
                nc.scalar.activation(e[:], lm[:], AF.Exp)
                ssum = stpool.tile([128, 1], fp32, tag="ssum")
                nc.vector.reduce_sum(ssum[:], e[:], axis=mybir.AxisListType.X)
                r = stpool.tile([128, 1], fp32, tag="r")
                nc.vector.reciprocal(r[:], ssum[:])
                att = spool.tile([128, T], bfl, tag="att")
                nc.vector.tensor_scalar_mul(att[:], e[:], r[:])

                # ---- transpose att -> [t, b] ----
                tp1 = auxpool.tile([128, 1024], bfl, tag="aux")
                nc.tensor.transpose(tp1[:, 0:128], att[:, 0:128], id_sb[:])
                aT1 = apool.tile([128, 128], bfl, tag="aT1")
                nc.vector.tensor_copy(aT1[:], tp1[:, 0:128])
                tp2 = auxpool.tile([128, 1024], bfl, tag="aux")
                nc.tensor.transpose(tp2[0:72, 0:128], att[:, 128:200],
                                    id_sb[:])
                aT2 = apool.tile([128, 128], bfl, tag="aT2")
                nc.vector.tensor_copy(aT2[0:72, :], tp2[0:72, 0:128])

                # ---- weighted sum: per-batch matmuls, 32 batches/psum
                # tile via col groups {0,32,64,96} x 8 free offsets ----
                wps = None
                for b in range(BT):
                    if b % 32 == 0:
                        wps = auxpool.tile([128, 512], fp32, tag="aux")
                    off = (b % 8) * D
                    cp = ((b % 32) // 8) * 32
                    nc.tensor.matmul(wps[cp:cp + 1, off:off + D],
                                     aT1[:, b:b + 1],
                                     v1[:, b * D:(b + 1) * D],
                                     start=True, stop=False,
                                     tile_position=(0, cp))
                    nc.tensor.matmul(wps[cp:cp + 1, off:off + D],
                                     aT2[0:72, b:b + 1],
                                     v2[0:72, b * D:(b + 1) * D],
                                     start=False, stop=True,
                                     tile_position=(0, cp))
                    if b % 32 == 31:
                        osb = opool.tile([128, 512], fp32, tag="osb")
                        if (b // 32) % 2 == 0:
                            nc.scalar.copy(osb[:], wps[:])
                        else:
                            nc.vector.tensor_copy(osb[:], wps[:])
                        bg = b0g + b - 31
                        src = osb[0:128:32, :].rearrange(
                            "p (b d) -> p b d", d=D)
                        dst = oD[bg:bg + 32, :].rearrange(
                            "(p b) d -> p b d", b=8)
                        nc.sync.dma_start(out=dst, in_=src)

            # Defer each B-tile's tail one iteration so the next B-tile's
            # MLP matmuls keep the PE busy while softmax/transpose run.
            for bt in range(NBT):
                mlp_block(bt)
                if bt >= 1:
                    tail_block(bt - 1)
            tail_block(NBT - 1)

    nc.compile()
    return nc


def _get_program():
    if "nc" not in _BUILT:
        _BUILT["nc"] = _build_program()
    return _BUILT["nc"]


def _prep_core(c, q, k, v, mask, W0, b0, W1, b1, Wf):
    s = slice(c * BC, (c + 1) * BC)
    qc = q[s]                      # [BC, 64] f32
    kc = k[s]                      # [BC, T, 64]
    vc = v[s]
    mc = mask[s]

    k2 = kc.reshape(N, D)
    feat = np.empty((128, N), dtype=bf16)
    feat[0:64] = k2.T.astype(bf16)
    feat[64:128] = (qc[:, None, :] * kc).reshape(N, D).T.astype(bf16)

    qb = np.empty((65, BC), dtype=bf16)
    qb[0:64] = qc.T.astype(bf16)
    qb[64] = bf16(1.0)

    A = (W0[0:64] + W0[128:192])
    C = (W0[64:128] - W0[128:192])
    P = W0[192:256]
    w0 = np.empty((128, 128), dtype=bf16)
    w0[0:64] = C.astype(bf16)
    w0[64:128] = P.astype(bf16)
    wA = np.empty((65, 128), dtype=bf16)
    wA[0:64] = A.astype(bf16)
    wA[64] = b0.astype(bf16)

    maskadd = np.where(mc == 0, np.float32(NEG), np.float32(0.0))

    return {
        "feat": feat,
        "qb": qb,
        "vv": vc.astype(bf16),
        "maskadd": maskadd.astype(np.float32),
        "w0": w0,
        "wA": wA,
        "w1": W1.astype(bf16),
        "wf2": np.vstack([Wf, Wf]).astype(bf16),
        "b1r": np.tile(b1.astype(np.float32), 2).reshape(128, 1),
        "ident": np.eye(128, dtype=np.float32).astype(bf16),
    }


def run(q, k, v, mask, W0, b0, W1, b1, Wf, bf, trace=False):
    from concourse.bass_utils import run_bass_kernel_spmd

    nc = _get_program()
    q = np.asarray(q, dtype=np.float32)
    k = np.asarray(k, dtype=np.float32)
    v = np.asarray(v, dtype=np.float32)
    mask = np.asarray(mask)
    in_maps = [
        _prep_core(c, q, k, v, mask,
                   np.asarray(W0, np.float32), np.asarray(b0, np.float32),
                   np.asarray(W1, np.float32), np.asarray(b1, np.float32),
                   np.asarray(Wf, np.float32))
        for c in range(NCORES)
    ]
    res = run_bass_kernel_spmd(nc, in_maps, list(range(NCORES)), trace=trace)
    out = np.concatenate([res.results[c]["o"] for c in range(NCORES)], axis=0)
    return np.ascontiguousarray(out.astype(np.float32)), res


def kernel(q, k, v, mask, W0, b0, W1, b1, Wf, bf):
    out, _ = run(q, k, v, mask, W0, b0, W1, b1, Wf, bf, trace=False)
    return out
